# revision 10
# baseline (speedup 1.0000x reference)
"""Bahdanau attention on 8 Trainium2 NeuronCores (Bass/Tile).

Reference computation (jax):
    enc_proj = einsum("sbe,de->bsd", encoder_outputs, We)
    dec_proj = (hidden @ Wd.T)[:, None, :]
    energy   = tanh(enc_proj + dec_proj)
    scores   = einsum("bsd,d->bs", energy, v)
    attw     = softmax(scores, axis=1)
    context  = einsum("bs,sbe->be", attw, encoder_outputs)
    returns (context, attw)

Sharding: data-parallel over batch. B=32 -> 4 batches per core;
We/Wd/v replicated. Each core is fully independent (softmax is per-batch).

Per-core kernel layout choices:
  * Main matmul computed as enc_projT[d, rows] so the decoder projection is a
    per-partition bias -> fused add+tanh in one ScalarE activation (PSUM->SBUF).
  * X tiles are loaded in natural [row, e] layout (contiguous DMA), transposed
    on the PE (128x128 transpose-mode matmuls) into [e, row] tiles for the
    main matmul, while the natural tiles stay resident in SBUF for the final
    context matmul -> encoder_outputs is read from HBM exactly once.
  * All matmuls run in float32r (full PE rate at moving-dim 512; plain fp32
    is 4 cycles/row).
"""

import numpy as np

S, B, E, D = 2048, 32, 1024, 1024
NCORES = 8
BC = B // NCORES  # 4 batches per core
P = 128
EC = E // P  # 8 contraction chunks
DC = D // P  # 8 d chunks
GROUP = 512  # rows processed per inner block (moving dim of main matmul)
NG = S // GROUP  # 4 groups per batch
RC = GROUP // P  # 4 row-chunks per group
NCH = S // P  # 16 row-chunks per batch

_CACHE = {}


def _build_bass():
    from contextlib import ExitStack

    import concourse.bass as bass
    import concourse.mybir as mybir
    import concourse.tile as tile
    from concourse import bacc
    from concourse.masks import make_identity

    f32 = mybir.dt.float32
    f32r = mybir.dt.float32r
    AF = mybir.ActivationFunctionType
    AX = mybir.AxisListType

    nc = bacc.Bacc("TRN2")

    enc = nc.dram_tensor("enc", [S, BC, E], f32, kind="ExternalInput")
    hidden = nc.dram_tensor("hidden", [BC, D], f32, kind="ExternalInput")
    We = nc.dram_tensor("We", [D, E], f32, kind="ExternalInput")
    Wd = nc.dram_tensor("Wd", [D, D], f32, kind="ExternalInput")
    v = nc.dram_tensor("v", [D], f32, kind="ExternalInput")
    ctx_out = nc.dram_tensor("ctx_out", [BC, E], f32, kind="ExternalOutput")
    attw_out = nc.dram_tensor("attw_out", [BC, S], f32, kind="ExternalOutput")

    def r(ap):
        return ap.bitcast(f32r)

    with tile.TileContext(nc) as tc, ExitStack() as ctx:
        singles = ctx.enter_context(tc.tile_pool(name="singles", bufs=1))
        xpool = ctx.enter_context(tc.tile_pool(name="x", bufs=1))
        xrawpool = ctx.enter_context(tc.tile_pool(name="xraw", bufs=3))
        xtpool = ctx.enter_context(tc.tile_pool(name="xt", bufs=2))
        epool = ctx.enter_context(tc.tile_pool(name="energy", bufs=3))
        spool = ctx.enter_context(tc.tile_pool(name="soft", bufs=2))
        ps_xt = ctx.enter_context(tc.tile_pool(name="ps_xt", bufs=2, space="PSUM"))
        ps_mm = ctx.enter_context(tc.tile_pool(name="ps_mm", bufs=2, space="PSUM"))
        ps_sc = ctx.enter_context(tc.tile_pool(name="ps_sc", bufs=1, space="PSUM"))
        ps_w = ctx.enter_context(tc.tile_pool(name="ps_w", bufs=1, space="PSUM"))
        ps_ctx = ctx.enter_context(tc.tile_pool(name="ps_ctx", bufs=1, space="PSUM"))

        identity = singles.tile([P, P], f32, tag="identity")
        make_identity(nc, identity)

        def absorb_ps(ps):
            # 1-element transpose that soaks up the fresh-PSUM-slot release
            # wait, so the real transposes carry at most one sync wait each
            # (TRN2 transpose-mode = LDW struct with a single wait slot).
            nc.tensor.transpose(ps[:1, 0, :1], identity[:1, :1], identity[:1, :1])

        # --- one-time setup: WeT, WdT, hiddenT, dec_projT, vT -------------
        # wet[p, ec, d] = We[d, ec*P + p]
        wet = singles.tile([P, EC, D], f32r, tag="wet")
        # vt[p, dc] = v[dc*P + p]
        vt_raw = singles.tile([P, DC], f32, tag="vt_raw")
        nc.sync.dma_start(out=vt_raw, in_=v.rearrange("(c p) -> p c", p=P))
        vt = singles.tile([P, DC], f32r, tag="vt")
        nc.vector.tensor_copy(out=vt, in_=vt_raw)
        dect = singles.tile([P, DC, BC], f32, tag="dect")

        with tc.tile_pool(name="setup", bufs=1) as setup_pool:
            # wdt[p, kc, d] = Wd[d, kc*P + p] -- setup-scoped, freed after
            wdt = setup_pool.tile([P, DC, D], f32r, tag="wdt")
            for w_nat_dram, w_t_sbuf in ((We, wet), (Wd, wdt)):
                for dcc in range(DC):
                    wnat = xrawpool.tile([P, E], f32, tag="xraw", name="wnat")
                    nc.sync.dma_start(
                        out=wnat, in_=w_nat_dram[dcc * P : (dcc + 1) * P, :]
                    )
                    for eh in range(2):
                        ps = ps_xt.tile([P, 4, P], f32, tag="xtp")
                        absorb_ps(ps)
                        for j in range(4):
                            ecc = eh * 4 + j
                            nc.tensor.transpose(
                                ps[:, j, :], wnat[:, ecc * P : (ecc + 1) * P], identity
                            )
                        nc.vector.tensor_copy(
                            out=w_t_sbuf[
                                :, eh * 4 : (eh + 1) * 4, dcc * P : (dcc + 1) * P
                            ],
                            in_=ps,
                        )

            # hiddenT: hidt[p, kc, b] = hidden[b, kc*P + p]
            hid_nat = setup_pool.tile([BC, D], f32, tag="hid_nat")
            nc.sync.dma_start(out=hid_nat, in_=hidden[:, :])
            hidt = setup_pool.tile([P, DC, BC], f32r, tag="hidt")
            for kc in range(DC):
                ps = ps_mm.tile([P, BC], f32, tag="mm")
                nc.tensor.transpose(ps[:1, :1], identity[:1, :1], identity[:1, :1])
                nc.tensor.transpose(
                    ps, hid_nat[:, kc * P : (kc + 1) * P], identity[:BC, :BC]
                )
                nc.vector.tensor_copy(out=hidt[:, kc, :], in_=ps)

            # dec_projT: dect[p, dc, b] = sum_k hidden[b, k] * Wd[dc*P + p, k]
            for dcc in range(DC):
                ps = ps_mm.tile([P, BC], f32, tag="mm")
                for kc in range(DC):
                    nc.tensor.matmul(
                        ps,
                        r(wdt[:, kc, dcc * P : (dcc + 1) * P]),
                        hidt[:, kc, :],
                        start=(kc == 0),
                        stop=(kc == DC - 1),
                    )
                nc.vector.tensor_copy(out=dect[:, dcc, :], in_=ps)

        # --- main loop over this core's batches ----------------------------
        for b in range(BC):
            xtiles = [
                xpool.tile([P, E], f32r, tag=f"x{i}", name=f"x{i}")
                for i in range(NCH)
            ]
            scores = spool.tile([1, S], f32, tag="scores")

            for g in range(NG):
                # xt[p, ec, row] = X[g*GROUP + row, ec*P + p]
                xt = xtpool.tile([P, EC, GROUP], f32r, tag="xt")
                for rc in range(RC):
                    xi = g * RC + rc
                    s0 = g * GROUP + rc * P
                    xraw = xrawpool.tile([P, E], f32, tag="xraw")
                    nc.sync.dma_start(out=xraw, in_=enc[s0 : s0 + P, b, :])
                    # rounded copy kept resident for the context matmul
                    nc.vector.tensor_copy(out=xtiles[xi], in_=xraw)
                    for eh in range(2):
                        ps = ps_xt.tile([P, 4, P], f32, tag="xtp")
                        absorb_ps(ps)
                        for j in range(4):
                            ecc = eh * 4 + j
                            nc.tensor.transpose(
                                ps[:, j, :],
                                xraw[:, ecc * P : (ecc + 1) * P],
                                identity,
                            )
                        nc.vector.tensor_copy(
                            out=xt[:, eh * 4 : (eh + 1) * 4, rc * P : (rc + 1) * P],
                            in_=ps,
                        )

                scps = ps_sc.tile([1, GROUP], f32, tag="scp")
                for dcc in range(DC):
                    mmps = ps_mm.tile([P, GROUP], f32, tag="mm")
                    for ec in range(EC):
                        nc.tensor.matmul(
                            mmps,
                            r(wet[:, ec, dcc * P : (dcc + 1) * P]),
                            xt[:, ec, :],
                            start=(ec == 0),
                            stop=(ec == EC - 1),
                        )
                    energy = epool.tile([P, GROUP], f32r, tag="energy")
                    nc.scalar.activation(
                        out=energy,
                        in_=mmps,
                        func=AF.Tanh,
                        bias=dect[:, dcc, b : b + 1],
                        scale=1.0,
                    )
                    nc.tensor.matmul(
                        scps,
                        vt[:, dcc : dcc + 1],
                        energy,
                        start=(dcc == 0),
                        stop=(dcc == DC - 1),
                    )
                nc.vector.tensor_copy(
                    out=scores[:, g * GROUP : (g + 1) * GROUP], in_=scps
                )

            # softmax over the full row [1, S]
            nmx = spool.tile([1, 1], f32, tag="nmx")
            nc.vector.reduce_max(out=nmx, in_=scores, axis=AX.X, negate=True)
            zsum = spool.tile([1, 1], f32, tag="zsum")
            nc.scalar.activation(
                out=scores,
                in_=scores,
                func=AF.Exp,
                bias=nmx,
                scale=1.0,
                accum_out=zsum,
            )
            rz = spool.tile([1, 1], f32, tag="rz")
            nc.vector.reciprocal(out=rz, in_=zsum)
            nc.vector.tensor_scalar_mul(out=scores, in0=scores, scalar1=rz)
            nc.sync.dma_start(out=attw_out[b : b + 1, :], in_=scores)

            # transpose weights into [row, chunk] layout for the context matmul
            wt_ps = ps_w.tile([P, NCH], f32, tag="wtp")
            nc.tensor.transpose(wt_ps[:1, :1], identity[:1, :1], identity[:1, :1])
            for c in range(NCH):
                nc.tensor.transpose(
                    wt_ps[:, c : c + 1],
                    scores[:, c * P : (c + 1) * P],
                    identity[:1, :1],
                )
            wt = spool.tile([P, NCH], f32r, tag="wt")
            nc.vector.tensor_copy(out=wt, in_=wt_ps)

            # context[e] = sum_row w[row] * X[row, e]
            cps = ps_ctx.tile([1, 2, GROUP], f32, tag="ctxp")
            for c in range(NCH):
                for h in range(2):
                    nc.tensor.matmul(
                        cps[:, h, :],
                        wt[:, c : c + 1],
                        r(xtiles[c][:, h * GROUP : (h + 1) * GROUP]),
                        start=(c == 0),
                        stop=(c == NCH - 1),
                    )
            ctx_sb = spool.tile([1, E], f32, tag="ctxsb")
            nc.vector.tensor_copy(
                out=ctx_sb.rearrange("a (h n) -> a h n", h=2), in_=cps
            )
            nc.sync.dma_start(out=ctx_out[b : b + 1, :], in_=ctx_sb)

    nc.compile()
    return nc


def get_nc():
    if "nc" not in _CACHE:
        _CACHE["nc"] = _build_bass()
    return _CACHE["nc"]


def make_in_maps(hidden, encoder_outputs, We, Wd, v):
    hidden = np.ascontiguousarray(np.asarray(hidden, dtype=np.float32))
    enc = np.ascontiguousarray(np.asarray(encoder_outputs, dtype=np.float32))
    We = np.ascontiguousarray(np.asarray(We, dtype=np.float32))
    Wd = np.ascontiguousarray(np.asarray(Wd, dtype=np.float32))
    v = np.ascontiguousarray(np.asarray(v, dtype=np.float32))

    in_maps = []
    for i in range(NCORES):
        b0 = i * BC
        in_maps.append(
            {
                "enc": np.ascontiguousarray(enc[:, b0 : b0 + BC, :]),
                "hidden": np.ascontiguousarray(hidden[b0 : b0 + BC, :]),
                "We": We,
                "Wd": Wd,
                "v": v,
            }
        )
    return in_maps


def kernel(hidden, encoder_outputs, We, Wd, v):
    from concourse.bass_utils import run_bass_kernel_spmd

    nc = get_nc()
    in_maps = make_in_maps(hidden, encoder_outputs, We, Wd, v)

    res = run_bass_kernel_spmd(nc, in_maps, list(range(NCORES)), trace=False)
    _CACHE["last_result"] = res

    context = np.concatenate(
        [res.results[i]["ctx_out"] for i in range(NCORES)], axis=0
    )
    attw = np.concatenate(
        [res.results[i]["attw_out"] for i in range(NCORES)], axis=0
    )
    return context, attw


# revision 24
# speedup vs baseline: 172.5172x; 172.5172x over previous
"""Bahdanau attention on 8 Trainium2 NeuronCores (Bass/Tile).

Reference computation (jax):
    enc_proj = einsum("sbe,de->bsd", encoder_outputs, We)
    dec_proj = (hidden @ Wd.T)[:, None, :]
    energy   = tanh(enc_proj + dec_proj)
    scores   = einsum("bsd,d->bs", energy, v)
    attw     = softmax(scores, axis=1)
    context  = einsum("bs,sbe->be", attw, encoder_outputs)
    returns (context, attw)

Sharding: data-parallel over batch. B=32 -> 4 batches per core;
We/Wd/v replicated. Each core is fully independent (softmax is per-batch).

Per-core kernel layout choices:
  * Main matmul computed as enc_projT[d, rows] so the decoder projection is a
    per-partition bias -> fused add+tanh in one ScalarE activation (PSUM->SBUF).
  * X tiles are loaded in natural [row, e] layout (contiguous DMA), transposed
    on the PE (128x128 transpose-mode matmuls) into [e, row] tiles for the
    main matmul, while the natural tiles stay resident in SBUF for the final
    context matmul -> encoder_outputs is read from HBM exactly once.
  * All matmuls run in float32r (full PE rate at moving-dim 512; plain fp32
    is 4 cycles/row).
"""

import numpy as np

S, B, E, D = 2048, 32, 1024, 1024
NCORES = 8
BC = B // NCORES  # 4 batches per core
P = 128
EC = E // P  # 8 contraction chunks
DC = D // P  # 8 d chunks
GROUP = 512  # rows processed per inner block (moving dim of main matmul)
NG = S // GROUP  # 4 groups per batch
RC = GROUP // P  # 4 row-chunks per group
NCH = S // P  # 16 row-chunks per batch

_CACHE = {}


def _build_bass(repeat=1):
    from contextlib import ExitStack

    import concourse.bass as bass
    import concourse.mybir as mybir
    import concourse.tile as tile
    from concourse import bacc
    from concourse.masks import make_identity

    f32 = mybir.dt.float32
    f32r = mybir.dt.float32r
    AF = mybir.ActivationFunctionType
    AX = mybir.AxisListType

    nc = bacc.Bacc("TRN2")

    enc = nc.dram_tensor("enc", [S, BC, E], f32, kind="ExternalInput")
    hidden = nc.dram_tensor("hidden", [BC, D], f32, kind="ExternalInput")
    We = nc.dram_tensor("We", [D, E], f32, kind="ExternalInput")
    Wd = nc.dram_tensor("Wd", [D, D], f32, kind="ExternalInput")
    v = nc.dram_tensor("v", [D], f32, kind="ExternalInput")
    ctx_out = nc.dram_tensor("ctx_out", [BC, E], f32, kind="ExternalOutput")
    attw_out = nc.dram_tensor("attw_out", [BC, S], f32, kind="ExternalOutput")

    def r(ap):
        return ap.bitcast(f32r)

    with tile.TileContext(nc) as tc, ExitStack() as ctx:
        singles = ctx.enter_context(tc.tile_pool(name="singles", bufs=1))
        stagepool = ctx.enter_context(tc.tile_pool(name="stage", bufs=5))
        ps_xt = ctx.enter_context(tc.tile_pool(name="ps_xt", bufs=2, space="PSUM"))
        ps_mm = ctx.enter_context(tc.tile_pool(name="ps_mm", bufs=2, space="PSUM"))
        ps_sc = ctx.enter_context(tc.tile_pool(name="ps_sc", bufs=2, space="PSUM"))
        ps_w = ctx.enter_context(tc.tile_pool(name="ps_w", bufs=1, space="PSUM"))
        ps_ctx = ctx.enter_context(tc.tile_pool(name="ps_ctx", bufs=1, space="PSUM"))

        identity = singles.tile([P, P], f32, tag="identity")
        make_identity(nc, identity)

        # --- one-time setup: WeT, WdT, hiddenT, dec_projT, vT -------------
        # wet[p, ec, d] = We[d, ec*P + p]
        wet = singles.tile([P, EC, D], f32r, tag="wet")
        # vt[p, dc] = v[dc*P + p]
        vt_raw = singles.tile([P, DC], f32, tag="vt_raw")
        nc.scalar.dma_start(out=vt_raw, in_=v.rearrange("(c p) -> p c", p=P))
        vt = singles.tile([P, DC], f32r, tag="vt")
        nc.vector.tensor_copy(out=vt, in_=vt_raw)
        dect = singles.tile([P, DC, BC], f32, tag="dect")
        ones_r = singles.tile([1, P], f32, tag="ones_r")
        nc.vector.memset(ones_r, 1.0)
        ones_c = singles.tile([P, 1], f32, tag="ones_c")
        nc.vector.memset(ones_c, 1.0)
        zrow = singles.tile([1, GROUP], f32, tag="zrow")
        nc.vector.memset(zrow, 0.0)

        with tc.tile_pool(name="setup", bufs=1) as setup_pool:
            # wdt[p, kc, d] = Wd[d, kc*P + p] -- setup-scoped, freed after
            wdt = setup_pool.tile([P, DC, D], f32r, tag="wdt")
            for w_nat_dram, w_t_sbuf in ((We, wet), (Wd, wdt)):
                for dcc in range(DC):
                    wnat = stagepool.tile([P, E], f32, tag="stage", name="wnat")
                    nc.scalar.dma_start(
                        out=wnat, in_=w_nat_dram[dcc * P : (dcc + 1) * P, :]
                    )
                    for eh in range(2):
                        ps = ps_xt.tile([P, 4, P], f32, tag="xtp")
                        for j in range(4):
                            ecc = eh * 4 + j
                            nc.tensor.transpose(
                                ps[:, j, :], wnat[:, ecc * P : (ecc + 1) * P], identity
                            )
                        nc.vector.tensor_copy(
                            out=w_t_sbuf[
                                :, eh * 4 : (eh + 1) * 4, dcc * P : (dcc + 1) * P
                            ],
                            in_=ps,
                        )

            # hiddenT: hidt[p, kc, b] = hidden[b, kc*P + p]
            hid_nat = setup_pool.tile([BC, D], f32, tag="hid_nat")
            nc.scalar.dma_start(out=hid_nat, in_=hidden[:, :])
            hidt = setup_pool.tile([P, DC, BC], f32r, tag="hidt")
            for kc in range(DC):
                ps = ps_mm.tile([P, BC], f32, tag="mm")
                nc.tensor.transpose(
                    ps, hid_nat[:, kc * P : (kc + 1) * P], identity[:BC, :BC]
                )
                nc.vector.tensor_copy(out=hidt[:, kc, :], in_=ps)

            # dec_projT: dect[p, dc, b] = sum_k hidden[b, k] * Wd[dc*P + p, k]
            for dcc in range(DC):
                ps = ps_mm.tile([P, BC], f32, tag="mm")
                for kc in range(DC):
                    nc.tensor.matmul(
                        ps,
                        r(wdt[:, kc, dcc * P : (dcc + 1) * P]),
                        hidt[:, kc, :],
                        start=(kc == 0),
                        stop=(kc == DC - 1),
                    )
                nc.vector.tensor_copy(out=dect[:, dcc, :], in_=ps)

        # main-loop-only pools, created after the setup scope frees its SBUF
        xpool = ctx.enter_context(tc.tile_pool(name="x", bufs=1))
        xtpool = ctx.enter_context(tc.tile_pool(name="xt", bufs=2))
        epool = ctx.enter_context(tc.tile_pool(name="energy", bufs=3))
        spool = ctx.enter_context(tc.tile_pool(name="soft", bufs=2))

        # --- main loop over this core's batches ----------------------------
        # (repeat > 1 re-runs the whole batch loop; used only for timing
        #  measurements -- the marginal time per repeat is the kernel time)
        for b in [bb for _ in range(repeat) for bb in range(BC)]:
            # xg[g][p, rc, e] = X[g*GROUP + rc*P + p, e]  (fp32r, resident;
            # feeds the context matmul at the end of the batch)
            xg = [
                xpool.tile([P, RC, E], f32r, tag=f"xg{g}", name=f"xg{g}")
                for g in range(NG)
            ]
            scores = spool.tile([1, S], f32, tag="scores")
            pmax = spool.tile([1, NG], f32, tag="pmax")
            wt_ps = ps_w.tile([P, NCH], f32, tag="wtp")

            for g in range(NG):
                # xt[p, ec, row] = X[g*GROUP + row, ec*P + p]
                xt = xtpool.tile([P, EC, GROUP], f32r, tag="xt")
                for h2 in range(2):
                    # one 1MB DMA covers two 128-row chunks
                    s0 = g * GROUP + h2 * 2 * P
                    stage = stagepool.tile([P, 2, E], f32, tag="stage")
                    nc.sync.dma_start(
                        out=stage,
                        in_=enc[s0 : s0 + 2 * P, b, :].rearrange(
                            "(c p) e -> p c e", p=P
                        ),
                    )
                    # rounded copy kept resident for the context matmul
                    nc.vector.tensor_copy(
                        out=xg[g][:, 2 * h2 : 2 * h2 + 2, :], in_=stage
                    )
                    for rcl in range(2):
                        rc = h2 * 2 + rcl
                        for eh in range(2):
                            ps = ps_xt.tile([P, 4, P], f32, tag="xtp")
                            for j in range(4):
                                ecc = eh * 4 + j
                                nc.tensor.transpose(
                                    ps[:, j, :],
                                    stage[:, rcl, ecc * P : (ecc + 1) * P],
                                    identity,
                                )
                            nc.vector.tensor_copy(
                                out=xt[
                                    :, eh * 4 : (eh + 1) * 4, rc * P : (rc + 1) * P
                                ],
                                in_=ps,
                            )

                scps = ps_sc.tile([1, GROUP], f32, tag="scp")
                for dcc in range(DC):
                    mmps = ps_mm.tile([P, GROUP], f32, tag="mm")
                    for ec in range(EC):
                        nc.tensor.matmul(
                            mmps,
                            r(wet[:, ec, dcc * P : (dcc + 1) * P]),
                            xt[:, ec, :],
                            start=(ec == 0),
                            stop=(ec == EC - 1),
                        )
                    energy = epool.tile([P, GROUP], f32r, tag="energy")
                    nc.scalar.activation(
                        out=energy,
                        in_=mmps,
                        func=AF.Tanh,
                        bias=dect[:, dcc, b : b + 1],
                        scale=1.0,
                    )
                    nc.tensor.matmul(
                        scps,
                        vt[:, dcc : dcc + 1],
                        energy,
                        start=(dcc == 0),
                        stop=(dcc == DC - 1),
                    )
                nc.vector.tensor_copy(
                    out=scores[:, g * GROUP : (g + 1) * GROUP], in_=scps
                )

            # softmax over the full row [1, S]
            nmx = spool.tile([1, 1], f32, tag="nmx")
            nc.vector.reduce_max(out=nmx, in_=scores, axis=AX.X, negate=True)
            zsum = spool.tile([1, 1], f32, tag="zsum")
            nc.scalar.activation(
                out=scores,
                in_=scores,
                func=AF.Exp,
                bias=nmx,
                scale=1.0,
                accum_out=zsum,
            )
            rz = spool.tile([1, 1], f32, tag="rz")
            nc.vector.reciprocal(out=rz, in_=zsum)
            nc.vector.tensor_scalar_mul(out=scores, in0=scores, scalar1=rz)
            nc.sync.dma_start(out=attw_out[b : b + 1, :], in_=scores)

            # transpose normalized weights into [row, chunk] layout
            wt_ps2 = ps_w.tile([P, NCH], f32, tag="wtp")
            for c in range(NCH):
                nc.tensor.transpose(
                    wt_ps2[:, c : c + 1],
                    scores[:, c * P : (c + 1) * P],
                    identity[:1, :1],
                )
            wt = spool.tile([P, NCH], f32r, tag="wt")
            nc.vector.tensor_copy(out=wt, in_=wt_ps2)

            # context[e] = sum_row w[row] * X[row, e]  (one PSUM bank,
            # the two 512-wide halves accumulated back to back)
            ctx_sb = spool.tile([1, E], f32, tag="ctxsb", bufs=1)
            for h in range(2):
                cps = ps_ctx.tile([1, GROUP], f32, tag="ctxp")
                for c in range(NCH):
                    xc = xg[c // RC][:, c % RC, :]
                    nc.tensor.matmul(
                        cps,
                        wt[:, c : c + 1],
                        xc[:, h * GROUP : (h + 1) * GROUP],
                        start=(c == 0),
                        stop=(c == NCH - 1),
                    )
                nc.vector.tensor_copy(
                    out=ctx_sb[:, h * GROUP : (h + 1) * GROUP], in_=cps
                )
            nc.sync.dma_start(out=ctx_out[b : b + 1, :], in_=ctx_sb)

    nc.compile()
    return nc


def get_nc(repeat=1):
    key = f"nc{repeat}"
    if key not in _CACHE:
        _CACHE[key] = _build_bass(repeat)
    return _CACHE[key]


def make_in_maps(hidden, encoder_outputs, We, Wd, v):
    hidden = np.ascontiguousarray(np.asarray(hidden, dtype=np.float32))
    enc = np.ascontiguousarray(np.asarray(encoder_outputs, dtype=np.float32))
    We = np.ascontiguousarray(np.asarray(We, dtype=np.float32))
    Wd = np.ascontiguousarray(np.asarray(Wd, dtype=np.float32))
    v = np.ascontiguousarray(np.asarray(v, dtype=np.float32))

    in_maps = []
    for i in range(NCORES):
        b0 = i * BC
        in_maps.append(
            {
                "enc": np.ascontiguousarray(enc[:, b0 : b0 + BC, :]),
                "hidden": np.ascontiguousarray(hidden[b0 : b0 + BC, :]),
                "We": We,
                "Wd": Wd,
                "v": v,
            }
        )
    return in_maps


def kernel(hidden, encoder_outputs, We, Wd, v):
    from concourse.bass_utils import run_bass_kernel_spmd

    nc = get_nc()
    in_maps = make_in_maps(hidden, encoder_outputs, We, Wd, v)

    res = run_bass_kernel_spmd(nc, in_maps, list(range(NCORES)), trace=False)
    _CACHE["last_result"] = res

    context = np.concatenate(
        [res.results[i]["ctx_out"] for i in range(NCORES)], axis=0
    )
    attw = np.concatenate(
        [res.results[i]["attw_out"] for i in range(NCORES)], axis=0
    )
    return context, attw
